# revision 9
# baseline (speedup 1.0000x reference)
"""TRN2 Bass kernel for nn_BipartiteLayer (bipartite GNN message passing).

Self-contained: host-side index preprocessing + weight algebra, an 8-core SPMD
Bass/Tile program (two owner-sorted edge passes, slot-aligned segment
reductions on PSUM, fused DVE max), and full-input/full-output wrapping.

Distribution: edges sharded by owner of the keyed node per pass; nodes
degree-sorted into 128-blocks; slot k holds each node's k-th edge aligned to
its partition rank. Gathered-side rows fetched by indirect DMA from a
replicated bf16 table built on-device (sharded compute + AllGather).
"""

from contextlib import ExitStack

import numpy as np
import ml_dtypes

import concourse.bass as bass
import concourse.bacc as bacc
import concourse.mybir as mybir
import concourse.tile as tile
from concourse.bass_utils import run_bass_kernel_spmd
from concourse.masks import make_identity

BF = ml_dtypes.bfloat16
P = 128
D = 128          # input dim
F = 256          # feature dim
OUT = 128        # output dim
HD = D + F + 4 * F  # 1408
KT = HD // P     # 11 k-tiles in output MLP
ROWC = 288       # bf16 cols per table row: 256 xp + 2 (f32 u/v bitcast) + pad (576B)
UV = 1e30        # pad-row u/v magnitude => score underflows to exactly 0

f32 = mybir.dt.float32
bf16 = mybir.dt.bfloat16
i32 = mybir.dt.int32


# ----------------------------------------------------------------- host side

def _pass_meta(key, other, Nn, No, n_cores):
    """Slot/block structure for one pass (key side owns the aggregation)."""
    per = Nn // n_cores
    padr = -(-(per + 1) // P) * P
    nblk = padr // P
    pero = No // n_cores
    padro = -(-(pero + 1) // P) * P
    PAD_O = padro - 1          # pad row id in the *other* (gathered) table
    PAD_OWN = padr - 1

    cores = []
    owner = key // per
    for c in range(n_cores):
        m = owner == c
        k_loc = (key[m] - c * per).astype(np.int64)
        o_ids = other[m].astype(np.int64)
        deg = np.bincount(k_loc, minlength=per)
        perm = np.argsort(-deg, kind="stable")
        rankof = np.empty(per, dtype=np.int64)
        rankof[perm] = np.arange(per)
        r = rankof[k_loc]
        order = np.argsort(r, kind="stable")
        r_s, o_s = r[order], o_ids[order]
        if len(r_s):
            starts = np.r_[0, np.flatnonzero(np.diff(r_s)) + 1]
            run_id = np.zeros(len(r_s), np.int64)
            run_id[starts[1:]] = 1
            run_id = np.cumsum(run_id)
            slot = np.arange(len(r_s)) - starts[run_id]
        else:
            slot = np.zeros(0, np.int64)
        deg_sorted = np.concatenate([deg[perm], np.zeros(padr - per, np.int64)])
        S_b = deg_sorted.reshape(nblk, P).max(1)
        cores.append(dict(r_s=r_s, o_s=o_s, slot=slot, perm=perm,
                          deg_sorted=deg_sorted))
        cores[-1]["S_b"] = S_b
    S_eq = np.maximum(np.stack([c["S_b"] for c in cores]).max(0), 1)
    C = int(S_eq.sum())
    col_base = np.zeros(nblk, np.int64)
    col_base[1:] = np.cumsum(S_eq[:-1])

    def other_row(ids):
        return (ids // pero) * padro + ids % pero

    for c, meta in enumerate(cores):
        gidx = np.full((P, C), PAD_O, np.int32)
        r_s, o_s, slot = meta["r_s"], meta["o_s"], meta["slot"]
        gidx[r_s % P, col_base[r_s // P] + slot] = other_row(o_s).astype(np.int32)
        oidx = np.full((P, nblk), PAD_OWN, np.int32)
        ranks = np.arange(per)
        oidx[ranks % P, ranks // P] = (meta["perm"] + c * padr).astype(np.int32)
        recip = (1.0 / np.maximum(meta["deg_sorted"], 1)).astype(np.float32)
        meta["gidx"] = gidx
        meta["oidx"] = oidx
        meta["recip"] = np.ascontiguousarray(recip.reshape(nblk, P).T)
    return dict(cores=cores, S_eq=S_eq.astype(int).tolist(), C=C, per=per,
                padr=padr, nblk=nblk, PAD_OWN=PAD_OWN)


def host_preprocess(inputs, n_cores):
    x_i = np.asarray(inputs["x_intt"], np.float32)
    x_m = np.asarray(inputs["x_mvtx"], np.float32)
    ei = np.asarray(inputs["edge_index"])
    W_i = np.asarray(inputs["W_in_intt"], np.float32)
    b_i = np.asarray(inputs["b_in_intt"], np.float32)
    W_m = np.asarray(inputs["W_in_mvtx"], np.float32)
    b_m = np.asarray(inputs["b_in_mvtx"], np.float32)
    Ws = np.asarray(inputs["W_score"], np.float32)
    bs = np.asarray(inputs["b_score"], np.float32)
    Wo_i = np.asarray(inputs["W_out_intt"], np.float32)
    bo_i = np.asarray(inputs["b_out_intt"], np.float32)
    Wo_m = np.asarray(inputs["W_out_mvtx"], np.float32)
    bo_m = np.asarray(inputs["b_out_mvtx"], np.float32)
    Ni, Nm = x_i.shape[0], x_m.shape[0]

    wu = W_i @ Ws[:F, 0]
    wv = W_m @ Ws[F:, 0]
    b_eff = float(b_i @ Ws[:F, 0] + b_m @ Ws[F:, 0] + bs[0])

    mi = _pass_meta(ei[0], ei[1], Ni, Nm, n_cores)
    mm = _pass_meta(ei[1], ei[0], Nm, Ni, n_cores)

    pad_row = np.zeros((1, ROWC), BF)
    pad_row[0, 256:258] = np.array([UV], np.float32).view(BF)  # u/v slot := 1e30

    def aug(W, b, wt):
        Wa = np.zeros((D, F + 1), np.float32)
        Wa[:, :F] = W
        Wa[:, F] = wt
        ba = np.zeros((1, F + 1), np.float32)
        ba[0, :F] = b
        return Wa.astype(BF), ba.astype(BF)

    Wa_i, ba_i = aug(W_i, b_i, wu)
    Wa_m, ba_m = aug(W_m, b_m, wv)

    def xpad(x, per, padr, c):
        out = np.zeros((padr, D), np.float32)
        out[:per] = x[c * per:(c + 1) * per]
        return out

    def xdeg(x, meta, c):
        per, padr = meta["per"], meta["padr"]
        out = np.zeros((padr, D), BF)
        out[:per] = x[c * per + meta["cores"][c]["perm"]].astype(BF)
        return out

    in_maps = []
    for c in range(n_cores):
        in_maps.append({
            "x_shard_intt": xpad(x_i, mi["per"], mi["padr"], c),
            "x_shard_mvtx": xpad(x_m, mm["per"], mm["padr"], c),
            "x_deg_intt": xdeg(x_i, mi, c),
            "x_deg_mvtx": xdeg(x_m, mm, c),
            "gidx_intt": mi["cores"][c]["gidx"],
            "gidx_mvtx": mm["cores"][c]["gidx"],
            "oidx_intt": mi["cores"][c]["oidx"],
            "oidx_mvtx": mm["cores"][c]["oidx"],
            "recip_intt": mi["cores"][c]["recip"],
            "recip_mvtx": mm["cores"][c]["recip"],
            "W_aug_intt": Wa_i, "b_aug_intt": ba_i,
            "W_aug_mvtx": Wa_m, "b_aug_mvtx": ba_m,
            "W_out_intt": Wo_i.astype(BF), "W_out_mvtx": Wo_m.astype(BF),
            "b_out_intt": bo_i.reshape(OUT, 1).astype(np.float32),
            "b_out_mvtx": bo_m.reshape(OUT, 1).astype(np.float32),
            "pad_row": pad_row,
        })
    cfg = dict(n_cores=n_cores, b_eff=b_eff, mi=mi, mm=mm)
    return cfg, in_maps


# --------------------------------------------------------------- bass program

def _phase0_side(tc, pools, side, x_shard, W_aug_sb, b_aug_sb, ones_sb,
                 identity_sb, stage, meta):
    """Compute this core's [padr, ROWC] slice of one side's gather table."""
    nc = tc.nc
    sb, psA, psB = pools["sb"], pools["psA"], pools["psB"]
    nblk = meta["nblk"]
    for s in range(nblk):
        xt = sb.tile([P, D], f32, tag="p0_x")
        nc.sync.dma_start(xt[:], x_shard[s * P:(s + 1) * P, :])
        xb = sb.tile([P, D], bf16, tag="p0_xb")
        nc.vector.tensor_copy(xb[:], xt[:])
        xT_ps = psA.tile([P, D], bf16, tag="p0_xT")
        nc.tensor.transpose(xT_ps[:], xb[:], identity_sb[:])
        xT = sb.tile([P, D], bf16, tag="p0_xTs")
        nc.scalar.copy(xT[:], xT_ps[:])
        xp_ps = psA.tile([P, F + 1], f32, tag="p0_xp")
        nc.tensor.matmul(xp_ps[:], lhsT=xT[:], rhs=W_aug_sb[:], start=True,
                         stop=False)
        nc.tensor.matmul(xp_ps[:], lhsT=ones_sb[:], rhs=b_aug_sb[:],
                         start=False, stop=True)
        st = sb.tile([P, ROWC], bf16, tag="p0_stage")
        nc.scalar.copy(st[:, 0:F], xp_ps[:, 0:F])
        nc.vector.tensor_copy(st[:, F:F + 2].bitcast(f32), xp_ps[:, F:F + 1])
        nc.gpsimd.memset(st[:, F + 2:ROWC], 0.0)
        nc.sync.dma_start(stage[s * P:(s + 1) * P, :], st[:])


def _pass_side(tc, pools, cfg_pass, own_first, own_tbl, gath_tbl, x_deg,
               gidx_in, oidx_in, recip_in, W_out_in, b_out_in, hT_out,
               identity_sb, beff_sb, smax):
    nc = tc.nc
    sb, psA, psB = pools["sb"], pools["psA"], pools["psB"]
    nblk = cfg_pass["nblk"]
    S_eq = cfg_pass["S_eq"]

    gidx_sb = sb.tile([P, cfg_pass["C"]], i32, tag="gidx")
    nc.sync.dma_start(gidx_sb[:], gidx_in[:])
    oidx_sb = sb.tile([P, nblk], i32, tag="oidx")
    nc.sync.dma_start(oidx_sb[:], oidx_in[:])
    recip_sb = sb.tile([P, nblk], f32, tag="recip")
    nc.sync.dma_start(recip_sb[:], recip_in[:])
    wout_sb = []
    for t in range(KT):
        w = sb.tile([P, OUT], bf16, tag=f"wout{t}")
        nc.sync.dma_start(w[:], W_out_in[t * P:(t + 1) * P, :])
        wout_sb.append(w)
    bout_sb = sb.tile([OUT, 1], f32, tag="bout")
    nc.sync.dma_start(bout_sb[:], b_out_in[:])

    o_mean = D + F + (0 if own_first else F)
    g_mean = D + F + (F if own_first else 0)
    o_max = D + 3 * F + (0 if own_first else F)
    g_max = D + 3 * F + (F if own_first else 0)

    ci = 0
    H_tiles = []
    for b in range(nblk):
        S = S_eq[b]
        own_t = sb.tile([P, ROWC], bf16, tag="own")
        nc.gpsimd.indirect_dma_start(
            out=own_t[:], out_offset=None, in_=own_tbl[:],
            in_offset=bass.IndirectOffsetOnAxis(ap=oidx_sb[:, b:b + 1], axis=0))
        H = sb.tile([P, HD], bf16, tag="H")
        nc.sync.dma_start(H[:, 0:D], x_deg[b * P:(b + 1) * P, :])
        nc.vector.tensor_copy(H[:, D:D + F], own_t[:, 0:F])

        strip = sb.tile([P, smax * ROWC], bf16, tag="strip")
        for k in range(S):
            nc.gpsimd.indirect_dma_start(
                out=strip[:, k * ROWC:(k + 1) * ROWC], out_offset=None,
                in_=gath_tbl[:],
                in_offset=bass.IndirectOffsetOnAxis(
                    ap=gidx_sb[:, ci + k:ci + k + 1], axis=0))

        scores = sb.tile([P, smax], f32, tag="scores")
        v_cols = strip[:].rearrange("p (s r) -> p s r", r=ROWC)[:, 0:S, F:F + 2]
        v_cols = v_cols.bitcast(f32).squeeze(-1)
        u_col = own_t[:, F:F + 2].bitcast(f32)
        nc.gpsimd.tensor_tensor(out=scores[:, 0:S], in0=v_cols,
                                in1=u_col.to_broadcast([P, S]),
                                op=mybir.AluOpType.add)
        nc.scalar.activation(scores[:, 0:S], scores[:, 0:S],
                             mybir.ActivationFunctionType.Abs, bias=beff_sb[:])
        nc.scalar.activation(scores[:, 0:S], scores[:, 0:S],
                             mybir.ActivationFunctionType.Exp, scale=-1.0)
        scores_bf = sb.tile([P, smax], bf16, tag="scores_bf")
        nc.scalar.copy(scores_bf[:, 0:S], scores[:, 0:S])

        nc.gpsimd.memset(H[:, g_max:g_max + F], 0.0)
        psum_g = psB.tile([P, F], f32, tag="psum_g")
        for k in range(S):
            rows = strip[:, k * ROWC:k * ROWC + F]
            diag = sb.tile([P, P], bf16, tag="diag")
            nc.gpsimd.affine_select(
                out=diag[:], in_=scores_bf[:, k:k + 1].to_broadcast([P, P]),
                compare_op=mybir.AluOpType.is_equal, fill=0.0, base=0,
                pattern=[[-1, P]], channel_multiplier=1)
            nc.tensor.matmul(psum_g[:], lhsT=diag[:], rhs=rows,
                             start=(k == 0), stop=(k == S - 1))
            nc.vector.scalar_tensor_tensor(
                out=H[:, g_max:g_max + F], in0=rows,
                scalar=scores[:, k:k + 1], in1=H[:, g_max:g_max + F],
                op0=mybir.AluOpType.mult, op1=mybir.AluOpType.max)
        ci += S

        s_sum = sb.tile([P, 1], f32, tag="s_sum")
        nc.vector.tensor_reduce(s_sum[:], scores[:, 0:S],
                                axis=mybir.AxisListType.X,
                                op=mybir.AluOpType.add)
        s_max = sb.tile([P, 1], f32, tag="s_max")
        nc.vector.tensor_reduce(s_max[:], scores[:, 0:S],
                                axis=mybir.AxisListType.X,
                                op=mybir.AluOpType.max)
        comb = sb.tile([P, 1], f32, tag="comb")
        nc.vector.tensor_tensor(out=comb[:], in0=s_sum[:],
                                in1=recip_sb[:, b:b + 1],
                                op=mybir.AluOpType.mult)
        nc.scalar.activation(H[:, o_mean:o_mean + F], own_t[:, 0:F],
                             mybir.ActivationFunctionType.Copy, scale=comb[:])
        nc.scalar.activation(H[:, g_mean:g_mean + F], psum_g[:],
                             mybir.ActivationFunctionType.Copy,
                             scale=recip_sb[:, b:b + 1])
        nc.scalar.activation(H[:, o_max:o_max + F], own_t[:, 0:F],
                             mybir.ActivationFunctionType.Relu,
                             scale=s_max[:])
        H_tiles.append(H)

        if b % 2 == 1 or b == nblk - 1:
            group = H_tiles
            H_tiles = []
            n = len(group)
            ht_bf = []
            for t in range(KT):
                hb = sb.tile([P, n * P], bf16, tag=f"ht{t}")
                for j, Hj in enumerate(group):
                    tp = psB.tile([P, P], bf16, tag="tp")
                    nc.tensor.transpose(tp[:], Hj[:, t * P:(t + 1) * P],
                                        identity_sb[:])
                    nc.scalar.copy(hb[:, j * P:(j + 1) * P], tp[:])
                ht_bf.append(hb)
            out_ps = psA.tile([P, n * P], f32, tag="out_ps")
            for t in range(KT):
                nc.tensor.matmul(out_ps[:], lhsT=wout_sb[t][:], rhs=ht_bf[t][:],
                                 start=(t == 0), stop=(t == KT - 1))
            hT_t = sb.tile([P, n * P], f32, tag="hT")
            nc.scalar.activation(hT_t[:], out_ps[:],
                                 mybir.ActivationFunctionType.Relu,
                                 bias=bout_sb[:])
            base = (b - n + 1) * P
            nc.sync.dma_start(hT_out[:, base:base + n * P], hT_t[:])


def build_program(cfg, n_cores):
    mi, mm = cfg["mi"], cfg["mm"]
    nc = bacc.Bacc("TRN2", target_bir_lowering=False, debug=False,
                   num_devices=n_cores)
    core_ids = list(range(n_cores))

    def din(name, shape, dt):
        return nc.dram_tensor(name, shape, dt, kind="ExternalInput")

    x_shard_i = din("x_shard_intt", [mi["padr"], D], f32)
    x_shard_m = din("x_shard_mvtx", [mm["padr"], D], f32)
    x_deg_i = din("x_deg_intt", [mi["padr"], D], bf16)
    x_deg_m = din("x_deg_mvtx", [mm["padr"], D], bf16)
    gidx_i = din("gidx_intt", [P, mi["C"]], i32)
    gidx_m = din("gidx_mvtx", [P, mm["C"]], i32)
    oidx_i = din("oidx_intt", [P, mi["nblk"]], i32)
    oidx_m = din("oidx_mvtx", [P, mm["nblk"]], i32)
    recip_i = din("recip_intt", [P, mi["nblk"]], f32)
    recip_m = din("recip_mvtx", [P, mm["nblk"]], f32)
    Wa_i = din("W_aug_intt", [D, F + 1], bf16)
    ba_i = din("b_aug_intt", [1, F + 1], bf16)
    Wa_m = din("W_aug_mvtx", [D, F + 1], bf16)
    ba_m = din("b_aug_mvtx", [1, F + 1], bf16)
    Wo_i = din("W_out_intt", [HD, OUT], bf16)
    Wo_m = din("W_out_mvtx", [HD, OUT], bf16)
    bo_i = din("b_out_intt", [OUT, 1], f32)
    bo_m = din("b_out_mvtx", [OUT, 1], f32)
    pad_row = din("pad_row", [1, ROWC], bf16)

    hT_i = nc.dram_tensor("hT_intt", [P, mi["padr"]], f32, kind="ExternalOutput")
    hT_m = nc.dram_tensor("hT_mvtx", [P, mm["padr"]], f32, kind="ExternalOutput")

    stage_i = nc.dram_tensor("stage_intt", [mi["padr"], ROWC], bf16)
    stage_m = nc.dram_tensor("stage_mvtx", [mm["padr"], ROWC], bf16)
    shared = {"addr_space": "Shared"} if n_cores > 4 else {}
    tbl_i = nc.dram_tensor("tbl_intt", [n_cores * mi["padr"], ROWC], bf16,
                           **shared)
    tbl_m = nc.dram_tensor("tbl_mvtx", [n_cores * mm["padr"], ROWC], bf16,
                           **shared)

    smax_i = max(mi["S_eq"])
    smax_m = max(mm["S_eq"])
    smax = max(smax_i, smax_m)

    with tile.TileContext(nc) as tc, ExitStack() as ctx:
        sb = ctx.enter_context(tc.tile_pool(name="sb", bufs=2))
        psA = ctx.enter_context(tc.tile_pool(name="psA", bufs=1, space="PSUM"))
        psB = ctx.enter_context(tc.tile_pool(name="psB", bufs=2, space="PSUM"))
        pools = {"sb": sb, "psA": psA, "psB": psB}

        identity_sb = sb.tile([P, P], bf16, tag="identity")
        make_identity(nc, identity_sb[:])
        ones_sb = sb.tile([1, P], bf16, tag="ones")
        nc.gpsimd.memset(ones_sb[:], 1.0)
        Wa_i_sb = sb.tile([D, F + 1], bf16, tag="WaI")
        nc.sync.dma_start(Wa_i_sb[:], Wa_i[:])
        ba_i_sb = sb.tile([1, F + 1], bf16, tag="baI")
        nc.sync.dma_start(ba_i_sb[:], ba_i[:])
        Wa_m_sb = sb.tile([D, F + 1], bf16, tag="WaM")
        nc.sync.dma_start(Wa_m_sb[:], Wa_m[:])
        ba_m_sb = sb.tile([1, F + 1], bf16, tag="baM")
        nc.sync.dma_start(ba_m_sb[:], ba_m[:])
        pad_sb = sb.tile([1, ROWC], bf16, tag="padrow")
        nc.sync.dma_start(pad_sb[:], pad_row[:])
        beff_sb = sb.tile([P, 1], f32, tag="beff")
        nc.gpsimd.memset(beff_sb[:], cfg["b_eff"])

        _phase0_side(tc, pools, "intt", x_shard_i, Wa_i_sb, ba_i_sb, ones_sb,
                     identity_sb, stage_i, mi)
        _phase0_side(tc, pools, "mvtx", x_shard_m, Wa_m_sb, ba_m_sb, ones_sb,
                     identity_sb, stage_m, mm)
        # pad row lives at local row padr-1; core 0's copy becomes the global
        # PAD row after AllGather. Write it on every core (harmless elsewhere).
        nc.sync.dma_start(stage_i[mi["padr"] - 1:mi["padr"], :], pad_sb[:])
        nc.sync.dma_start(stage_m[mm["padr"] - 1:mm["padr"], :], pad_sb[:])

        nc.gpsimd.collective_compute(
            "AllGather", mybir.AluOpType.bypass,
            replica_groups=[core_ids], ins=[stage_i[:]], outs=[tbl_i[:]])
        nc.gpsimd.collective_compute(
            "AllGather", mybir.AluOpType.bypass,
            replica_groups=[core_ids], ins=[stage_m[:]], outs=[tbl_m[:]])

        _pass_side(tc, pools, mi, True, tbl_i, tbl_m, x_deg_i, gidx_i, oidx_i,
                   recip_i, Wo_i, bo_i, hT_i, identity_sb, beff_sb, smax)
        _pass_side(tc, pools, mm, False, tbl_m, tbl_i, x_deg_m, gidx_m, oidx_m,
                   recip_m, Wo_m, bo_m, hT_m, identity_sb, beff_sb, smax)

    nc.compile()
    return nc


# ------------------------------------------------------------------- wrapper

_CACHE = {}
TRACE = False
LAST = {}


def kernel(**inputs):
    n_cores = 8
    ei = np.asarray(inputs["edge_index"])
    key = (inputs["x_intt"].shape, inputs["x_mvtx"].shape, ei.shape,
           hash(ei.tobytes()))
    if key not in _CACHE:
        cfg, in_maps = host_preprocess(inputs, n_cores)
        nc = build_program(cfg, n_cores)
        _CACHE.clear()
        _CACHE[key] = (cfg, nc)
    else:
        cfg, nc = _CACHE[key]
        _, in_maps = host_preprocess(inputs, n_cores)

    res = run_bass_kernel_spmd(nc, in_maps, list(range(n_cores)), trace=TRACE)
    LAST["res"] = res
    mi, mm = cfg["mi"], cfg["mm"]
    Ni = mi["per"] * n_cores
    Nm = mm["per"] * n_cores
    h_i = np.zeros((Ni, OUT), np.float32)
    h_m = np.zeros((Nm, OUT), np.float32)
    for c in range(n_cores):
        out = res.results[c]
        hi = out["hT_intt"].T
        hm = out["hT_mvtx"].T
        h_i[c * mi["per"] + mi["cores"][c]["perm"]] = hi[:mi["per"]]
        h_m[c * mm["per"] + mm["cores"][c]["perm"]] = hm[:mm["per"]]
    return h_i, h_m
